# revision 12
# baseline (speedup 1.0000x reference)
"""GAT-style GNN message passing on 8 TRN2 NeuronCores.

Math: with LEAK=1 the leaky-relu is identity, so
  e[i,j,h] = e_src[i,h] + e_dst[j,h]
and softmax over j cancels e_src (and any row max) exactly:
  attn[i,j,h] = adj[i,j]*exp(e_dst[j,h]) / sum_j adj[i,j]*exp(e_dst[j,h])
  out[i,(h,f)] = (adj @ (z*h))[i,(h,f)] / (adj @ z)[i,h],  z = exp(e_dst)
then elu + log_softmax per row. log_softmax is shift invariant, so
elu(x) is computed as relu(x) + exp(min(x,0)) (drops the uniform -1),
and no max subtraction is needed (y is bounded in [e^-10, ~10]).

Sharding: rows (query nodes) of adj/out across 8 cores. x is row-sharded
too; each core computes its local h slab and the G=[z*h | z] slab for its
512 nodes, then pushes it straight into every peer's SBUF with
remote_dma_broadcast (one single-slot broadcast per peer, XOR-relative
destinations). This replaces the NRT AllGather of the previous version,
whose software path cost ~70us of the 109us runtime. A 1-byte prelude
AllGather (bir_kernel_barrier) guarantees no remote write lands before
the peer's kernel has cleared its semaphores; the receiver waits on a
remote semaphore (2 increments per arriving slab, 14 total) before the
first matmul that consumes a peer slab.

The XOR-relative destination routing works in *physical* core ids; the
driver's logical->physical map was probed once on this machine
(probe_rdma.py) and is hardcoded in SLOT_MAP: slot k of core r holds the
G slab of logical core SLOT_MAP[r][k]. The host permutes each core's
adjacency column blocks into slot order, so the device program is
rank-oblivious.

Precision: the aggregation matmul adj @ G runs in fp8e4 (e4m3) DoubleRow
mode at 2x bf16 PE rate. adj entries 0/1 are exact in fp8; G is sent as
an fp8 hi/lo split (G = hi + lo, both accumulated into fp32 PSUM), which
carries ~8 effective mantissa bits (bf16-level, ~4e-3 relative on the
attention-averaged output, far inside the 2e-2 gate). The h = x@W_ext
matmul runs in bf16 (x and W rounded), fp32 PSUM accumulation.

All DRAM->SBUF tensors use partition-major host layouts ([128, ...],
one contiguous run per partition).

Per-core device program (R = N/8 = 512 rows, P=128):
  inputs:  xt [128, KC*R] bf16   xt[p, kc*R+r]  = x[c*R+r, kc*128+p]
           wt [128, KC*72] bf16  wt[p, kc*72+e] = w_ext[kc*128+p, e]
                                 (w_ext = [W | W @ blockdiag-reduced a_dst])
           at [128, NC*R] fp8    at[p, (k*RC+q)*R+r] =
                                   adj[c*R+r, (SLOT_MAP[c][k]*RC+q)*128+p]
  output:  out_p [128, RC*64]    out_p[p, q*64+f] = out[q*128+p, f]
"""

import sys

import numpy as np

if "/opt/trn_rl_repo" not in sys.path:
    sys.path.insert(0, "/opt/trn_rl_repo")

import ml_dtypes  # noqa: E402

import concourse.bass as bass  # noqa: E402
import concourse.tile as tile  # noqa: E402
from concourse import bacc, mybir  # noqa: E402
from concourse.bass_utils import run_bass_kernel_spmd  # noqa: E402
from concourse.masks import make_identity  # noqa: E402

N_CORES = 8
H = 8
F = 8
HF = H * F  # 64
EXT = HF + H  # 72: [g | z]
K_IN = 1024
P = 128

FP32 = mybir.dt.float32
BF16 = mybir.dt.bfloat16
FP8 = mybir.dt.float8e4
AFT = mybir.ActivationFunctionType
ALU = mybir.AluOpType
DR = mybir.MatmulPerfMode.DoubleRow

# slot->logical-sender map for the XOR-relative remote broadcasts, probed
# on this machine (driver swaps logical cores {4,5} <-> {6,7} physically).
SLOT_MAP = [
    [0, 1, 2, 3, 6, 7, 4, 5],
    [1, 0, 3, 2, 7, 6, 5, 4],
    [2, 3, 0, 1, 4, 5, 6, 7],
    [3, 2, 1, 0, 5, 4, 7, 6],
    [4, 5, 6, 7, 2, 3, 0, 1],
    [5, 4, 7, 6, 3, 2, 1, 0],
    [6, 7, 4, 5, 0, 1, 2, 3],
    [7, 6, 5, 4, 1, 0, 3, 2],
]


def _bcast_f(ap_in, n_free):
    """Append a stride-0 axis of size n_free to an AP (broadcast)."""
    return bass.AP(
        tensor=ap_in.tensor,
        offset=ap_in.offset,
        ap=[*ap_in.ap, [0, n_free]],
    )


def build_bass(n_nodes: int) -> bass.Bass:
    R = n_nodes // N_CORES
    KC = K_IN // P  # k-chunks for the h matmul
    NC = n_nodes // P  # j-chunks for the aggregation matmul
    RC = R // P  # 128-row output chunks per core
    assert R % P == 0 and RC % 2 == 0

    nc = bacc.Bacc(num_devices=N_CORES)

    xt = nc.declare_dram_parameter("xt", [P, KC * R], BF16, isOutput=False)
    at = nc.declare_dram_parameter("at", [P, NC * R], FP8, isOutput=False)
    wt = nc.declare_dram_parameter("wt", [P, KC * EXT], BF16, isOutput=False)
    out = nc.declare_dram_parameter("out", [P, RC * HF], FP32, isOutput=True)

    # Pin the cross-core sems to the top of the kernel range so they never
    # collide with Tile's own (bottom-up) allocations.
    rsem = nc.alloc_semaphore("g_rsem")
    lsem = nc.alloc_semaphore("g_lsem")

    with tile.TileContext(nc) as tc:
        with (
            tc.tile_pool(name="singles", bufs=1) as singles,
            tc.tile_pool(name="bigpsum", bufs=2, space="PSUM") as bigpsum,
            tc.tile_pool(name="chunkps", bufs=4, space="PSUM") as chunkps,
        ):
            ident = singles.tile([P, P], FP32)
            make_identity(nc, ident)

            # --- loads (p-major, one contiguous run per partition) ---
            w_sb = singles.tile([P, KC, EXT], BF16)
            nc.sync.dma_start(
                out=w_sb, in_=wt[:].rearrange("p (c e) -> p c e", c=KC)
            )
            xt_sb = singles.tile([P, KC, R], BF16)
            xt_view = xt[:].rearrange("p (c r) -> p c r", c=KC)
            nc.sync.dma_start(out=xt_sb[:, : KC // 2, :], in_=xt_view[:, : KC // 2, :])
            nc.sync.dma_start(out=xt_sb[:, KC // 2 :, :], in_=xt_view[:, KC // 2 :, :])
            at_sb = singles.tile([P, NC, R], FP8)
            at_view = at[:].rearrange("p (n r) -> p n r", n=NC)
            nc.sync.dma_start(out=at_sb[:, : NC // 2, :], in_=at_view[:, : NC // 2, :])
            nc.sync.dma_start(out=at_sb[:, NC // 2 :, :], in_=at_view[:, NC // 2 :, :])

            # --- hT = w_ext.T @ x_loc.T : [EXT, R] bf16 inputs, fp32 acc ---
            hT_ps = bigpsum.tile([EXT, R], FP32, tag="bigps", name="hT")
            for c in range(KC):
                nc.tensor.matmul(
                    hT_ps,
                    lhsT=w_sb[:, c, :],
                    rhs=xt_sb[:, c, :],
                    start=(c == 0),
                    stop=(c == KC - 1),
                )
            hT_sb = singles.tile([EXT, R], FP32)
            nc.vector.tensor_copy(hT_sb, hT_ps)

            # --- transpose h chunks to row-major [P, RC, EXT] ---
            h_sb = singles.tile([P, RC, EXT], FP32)
            for q in range(RC):
                t_ps = chunkps.tile([P, EXT], FP32, tag="cps", name=f"t{q}")
                nc.tensor.transpose(
                    t_ps, hT_sb[:, q * P : (q + 1) * P], ident[:EXT, :EXT]
                )
                nc.vector.tensor_copy(h_sb[:, q, :], t_ps)

            # --- G = [h*z | z], z = exp(e_dst); fp8 hi/lo into slot 0 ---
            # g_all[p, slot, s, q, e]: s=0 hi, s=1 lo; slot k = peer slab.
            g_all = singles.tile([P, N_CORES, 2, RC, EXT], FP8)
            z_sb = singles.tile([P, RC, H], FP32)
            nc.scalar.activation(z_sb, h_sb[:, :, HF:EXT], AFT.Exp)
            g_sb = singles.tile([P, RC, EXT], FP32)
            nc.vector.tensor_mul(
                g_sb[:, :, 0:HF].rearrange("p q (h f) -> p q h f", h=H),
                h_sb[:, :, 0:HF].rearrange("p q (h f) -> p q h f", h=H),
                _bcast_f(z_sb, F),
            )
            nc.vector.tensor_copy(g_sb[:, :, HF:EXT], z_sb)
            nc.vector.tensor_copy(g_all[:, 0, 0], g_sb)  # hi (fp8)
            hi32 = singles.tile([P, RC, EXT], FP32)
            nc.vector.tensor_copy(hi32, g_all[:, 0, 0])
            lo32 = singles.tile([P, RC, EXT], FP32)
            nc.vector.tensor_sub(lo32, g_sb, hi32)
            nc.vector.tensor_copy(g_all[:, 0, 1], lo32)  # lo (fp8)

            # --- push my slab into every peer's slot k (XOR-relative) ---
            preps = []
            for k in range(1, N_CORES):
                rd = [None] * 8
                rd[k] = (0, k)
                preps.append(
                    nc.gpsimd.remote_dma_broadcast(
                        out_ap=g_all[:, k],
                        in_ap=g_all[:, 0],
                        remote_sem=rsem,
                        local_sem=lsem,
                        rdests=rd,
                    )
                )
            nc.gpsimd.trigger_dma(count=None)

            # --- aggregate: outT += G_k.T @ adjT_k (fp8, fp32 PSUM acc) ---
            outT_ps = bigpsum.tile([EXT, R], FP32, tag="bigps", name="outT")
            first_remote_mm = None
            n_mm = N_CORES * 2 * RC
            i_mm = 0
            for k in range(N_CORES):
                for s in range(2):
                    for q in range(RC):
                        mm = nc.tensor.matmul(
                            outT_ps,
                            lhsT=g_all[:, k, s, q, :],
                            rhs=at_sb[:, k * RC + q, :],
                            start=(i_mm == 0),
                            stop=(i_mm == n_mm - 1),
                        )
                        if first_remote_mm is None and k == 1:
                            first_remote_mm = mm
                        i_mm += 1
            outT_sb = singles.tile([EXT, R], FP32)
            nc.vector.tensor_copy(outT_sb, outT_ps)

            # --- postprocess: transpose, div, elu(+1), log_softmax ---
            o_sb = singles.tile([P, RC, EXT], FP32)
            for q in range(RC):
                o_ps = chunkps.tile([P, EXT], FP32, tag="cps", name=f"o{q}")
                nc.tensor.transpose(
                    o_ps, outT_sb[:, q * P : (q + 1) * P], ident[:EXT, :EXT]
                )
                nc.vector.tensor_copy(o_sb[:, q, :], o_ps)
            rd_sb = singles.tile([P, RC, H], FP32)
            nc.vector.reciprocal(rd_sb, o_sb[:, :, HF:EXT])
            xo = singles.tile([P, RC, HF], FP32)
            nc.vector.tensor_mul(
                xo[:].rearrange("p q (h f) -> p q h f", h=H),
                o_sb[:, :, 0:HF].rearrange("p q (h f) -> p q h f", h=H),
                _bcast_f(rd_sb, F),
            )
            # y = relu(xo) + exp(min(xo, 0))  (= elu + 1; log_softmax shift-safe)
            mo = singles.tile([P, RC, HF], FP32)
            nc.vector.tensor_scalar_min(mo, xo, 0.0)
            eo = singles.tile([P, RC, HF], FP32)
            nc.scalar.activation(eo, mo, AFT.Exp)
            yo = singles.tile([P, RC, HF], FP32)
            nc.vector.scalar_tensor_tensor(
                out=yo, in0=xo, scalar=0.0, in1=eo, op0=ALU.max, op1=ALU.add
            )
            ex = singles.tile([P, RC, HF], FP32)
            nc.scalar.activation(ex, yo, AFT.Exp)
            sm = singles.tile([P, RC, 1], FP32)
            nc.vector.reduce_sum(sm, ex, axis=mybir.AxisListType.X)
            ls = singles.tile([P, RC, 1], FP32)
            nc.scalar.activation(ls, sm, AFT.Ln)
            out_sb = singles.tile([P, RC, HF], FP32)
            nc.vector.tensor_sub(
                out_sb,
                yo,
                _bcast_f(ls[:, :, 0], HF),
            )
            nc.sync.dma_start(out=out[:].rearrange("p (q f) -> p q f", q=RC), in_=out_sb)

    # --- post-scheduling splices. The tile scheduler's single-core sim
    # cannot model cross-core semaphore increments (it would report a
    # deadlock), so these waits are inserted into the engine streams after
    # scheduling, before Bacc.compile legalizes waits. ---
    # 1) barrier wait on gpsimd before the first remote prep: no send may
    #    fire before every peer has started (prelude AllGather + then_inc).
    nc._bir_kernel_barrier_sem_replica_groups.append(set(range(N_CORES)))
    bw = nc.gpsimd.wait_ge(nc._bir_kernel_barrier_sem, nc.bir_kernel_barrier_sem_inc)
    # 2) arrival wait on the tensor engine before the first matmul that
    #    consumes a peer slab (7 slabs x 2 increments each).
    wi = nc.tensor.wait_ge(rsem, 14)

    def _splice_before(winst, target):
        src = dst = None
        for f in nc.m.functions:
            for b in f.blocks:
                if winst.ins in b.instructions:
                    src = b
                if target.ins in b.instructions:
                    dst = b
        assert src is not None and dst is not None
        src.instructions.remove(winst.ins)
        dst.instructions.insert(dst.instructions.index(target.ins), winst.ins)

    _splice_before(bw, preps[0])
    _splice_before(wi, first_remote_mm)

    # 3) Semaphores are persistent physical device state (no preamble clear
    # runs when target_bir_lowering=False), so a previous NEFF's leftovers
    # would corrupt Tile's waits and the rsem arrival count. Clear the whole
    # allocatable kernel-sem range at entry, BEFORE the prelude-barrier
    # collective posts: peers cannot send until this core's contribution is
    # in, which is strictly after the clear.
    nc.insert_bir_kernel_barrier_sem_inc()
    sc = nc.gpsimd.sem_clear(range(154, 256))
    entry = nc.m.functions[0].blocks[0]
    cc_idx = None
    for i, ins in enumerate(entry.instructions):
        if isinstance(ins, mybir.InstCollectiveCompute):
            cc_idx = i
            break
    assert cc_idx is not None, "prelude barrier collective not found"
    for winst in (sc,):
        src = None
        for f in nc.m.functions:
            for b in f.blocks:
                if winst.ins in b.instructions:
                    src = b
        assert src is not None
        src.instructions.remove(winst.ins)
        entry.instructions.insert(cc_idx, winst.ins)

    # Force all ACT activations (Exp + Ln) onto the one table set containing
    # both, so only ONE ACT_TABLE_LOAD is emitted (early, hidden under DMA)
    # instead of a ~1.3us reload at every Exp<->Ln switch. Set indices must
    # stay aligned with act_info.json, so empty the other sets rather than
    # filtering the list.
    orig_gat = bacc.get_activation_tables

    def _one_set(arch):
        return {
            k: (v if k == "natural_log_exp_and_others" else set())
            for k, v in orig_gat(arch).items()
        }

    bacc.get_activation_tables = _one_set
    try:
        nc.finalize()
    finally:
        bacc.get_activation_tables = orig_gat
    return nc


def _pmajor(a, chunk):
    """[chunk*P, L] -> [P, chunk*L] partition-major layout."""
    n, L = a.shape[0] // P, a.shape[1]
    return np.ascontiguousarray(
        a.reshape(n, P, L).transpose(1, 0, 2).reshape(P, n * L)
    )


def _host_prep(x, adj, W, a_dst, n_nodes):
    """Build per-core input maps."""
    R = n_nodes // N_CORES
    RC = R // P
    Wd = np.einsum(
        "khf,hf->kh", W.reshape(K_IN, H, F), a_dst, dtype=np.float32
    ).astype(np.float32)
    w_ext = np.concatenate([W, Wd], axis=1).astype(ml_dtypes.bfloat16)  # [1024, 72]
    wt = _pmajor(w_ext, K_IN // P)
    adj_f8 = adj.astype(ml_dtypes.float8_e4m3)  # exact for 0/1
    in_maps = []
    for c in range(N_CORES):
        rows = slice(c * R, (c + 1) * R)
        # adjacency slab with column node-blocks permuted into slot order:
        # at[p, (k*RC+q)*R + r] = adj[c*R+r, (SLOT_MAP[c][k]*RC+q)*128 + p]
        A = adj_f8[rows].reshape(R, N_CORES, RC, P)
        A = A[:, SLOT_MAP[c], :, :]  # slot-major node blocks
        at_c = np.ascontiguousarray(
            A.transpose(3, 1, 2, 0).reshape(P, n_nodes // P * R)
        )
        in_maps.append(
            {
                "xt": _pmajor(
                    np.ascontiguousarray(
                        x[rows].T.astype(ml_dtypes.bfloat16)
                    ),
                    K_IN // P,
                ),
                "at": at_c,
                "wt": wt,
            }
        )
    return in_maps


_BUILT = {}


def run(x, adj, W, a_dst, trace=False):
    n_nodes = x.shape[0]
    R = n_nodes // N_CORES
    RC = R // P
    if n_nodes not in _BUILT:
        _BUILT[n_nodes] = build_bass(n_nodes)
    nc = _BUILT[n_nodes]
    in_maps = _host_prep(x, adj, W, a_dst, n_nodes)
    res = run_bass_kernel_spmd(
        nc, in_maps, list(range(N_CORES)), trace=trace
    )
    blocks = []
    for c in range(N_CORES):
        o = res.results[c]["out"]  # [P, RC*HF] p-major
        blocks.append(
            o.reshape(P, RC, HF).transpose(1, 0, 2).reshape(R, HF)
        )
    return np.concatenate(blocks, axis=0).astype(np.float32), res


def kernel(x, adj, W, a_src, a_dst):
    x = np.asarray(x, dtype=np.float32)
    adj = np.asarray(adj)
    W = np.asarray(W, dtype=np.float32)
    a_dst = np.asarray(a_dst, dtype=np.float32)
    out, _ = run(x, adj, W, a_dst, trace=False)
    return out
